# revision 9
# baseline (speedup 1.0000x reference)
"""Non-overlapping Conv1d (kernel=2, stride=2) on 8 TRN2 NeuronCores.

out[b, o, p] = sum_{c,k} x[b, c, 2p+k] * w[o, c, k] / sqrt(cin)

Strategy: data-parallel over batch (4 batches per core), weight replicated.
Per batch: out[b] = W0 @ xe + W1 @ xo with the contraction over cin=128 on
the partition dim; xe/xo are the even/odd phases of x, deinterleaved on
the host so every on-chip access is contiguous.

Precision/traffic: the kernel is HBM-bound, so x is sent as fp8e3 (e3m4,
4 mantissa bits, 1 byte) and fed STRAIGHT into the PE as the moving
operand against bf16 stationary weights (mixed-dtype matmul measured
exact on HW, 1 col/cycle).  No on-chip dequant pass is needed.  Output is
stored bf16 and upconverted on the host.  End-to-end L2 error ~1.3e-2,
inside the 2e-2 gate.

Per-core HBM traffic: 4.19 MB x (fp8) + 4.19 MB out (bf16) = 8.39 MB.

Engines: SP ring (nc.sync) carries x loads + last-batch stores; ACT ring
carries early stores; weights ride the SWDGE (gpsimd) ring so they land
before the first matmul without delaying the x stream.  PSUM->SBUF
copies (paired banks, FD=1024) alternate DVE/ACT.
"""

import math
from contextlib import ExitStack

import numpy as np
import ml_dtypes

import concourse.bass as bass
import concourse.mybir as mybir
import concourse.tile as tile
from concourse import bacc
from concourse.bass_utils import run_bass_kernel_spmd

# Problem shape (hardcoded per contract)
BS, CIN, D = 32, 128, 8192
COUT = 128
N_CORES = 8
B_PER_CORE = BS // N_CORES          # 4
P_OUT = D // 2                      # 4096 output positions per (b, o)
PSUM_N = 512                        # fp32 PSUM bank limit = matmul free dim
COPY_N = 1024                       # positions per PSUM->SBUF copy (2 banks)

# per-batch chunk plans (positions): batch 0 starts fine-grained so the
# pipeline primes fast; the last batch's stores go per-COPY_N for a short
# tail
CHUNK_PLAN = [
    [1024, 1024, 2048],
    [4096],
    [4096],
    [4096],
]

_cache = {}


def _build():
    nc = bacc.Bacc("TRN2", target_bir_lowering=False, debug=False, num_devices=N_CORES)
    f32 = mybir.dt.float32
    bf16 = mybir.dt.bfloat16
    e3 = mybir.dt.float8e3

    x_d = nc.dram_tensor(
        "xq", [B_PER_CORE, CIN, 2, P_OUT], e3, kind="ExternalInput"
    ).ap()
    w_d = nc.dram_tensor("wT", [CIN, 2, COUT], bf16, kind="ExternalInput").ap()
    out_d = nc.dram_tensor(
        "out", [B_PER_CORE, COUT, P_OUT], bf16, kind="ExternalOutput"
    ).ap()

    with tile.TileContext(nc) as tc, ExitStack() as ctx:
        wpool = ctx.enter_context(tc.tile_pool(name="w", bufs=1))
        xpool = ctx.enter_context(tc.tile_pool(name="x", bufs=4))
        opool = ctx.enter_context(tc.tile_pool(name="o", bufs=4))
        ppool = ctx.enter_context(tc.tile_pool(name="p", bufs=4, space="PSUM"))

        # Weights first on the SP ring (host already stores them [c, k, o]
        # so the transfer is one contiguous 512B line per partition); they
        # land ~1 us after the ring warms, before the first matmul.
        w_t = wpool.tile([CIN, 2, COUT], bf16)
        nc.sync.dma_start(w_t[:], w_d)

        nco = 0  # running copy counter for DVE/ACT alternation
        for b in range(B_PER_CORE):
            pos = 0
            last_b = b == B_PER_CORE - 1
            for cp in CHUNK_PLAN[b]:
                x_t = xpool.tile([CIN, 2, 4096], e3, tag="x")
                # batch 2 loads ride the SWDGE (gpsimd) ring: a third DMA
                # stream alongside SP loads and ACT stores raises total
                # HBM read bandwidth
                ld_eng = nc.gpsimd if b == 2 else nc.sync
                ld_eng.dma_start(
                    x_t[:, :, :cp], x_d[b, :, :, pos:pos + cp]
                )
                o_t = opool.tile([COUT, 4096], bf16, tag="o")
                for jc in range(cp // COPY_N):
                    acc = ppool.tile([COUT, COPY_N], f32, name="acc")
                    for jj in range(COPY_N // PSUM_N):
                        fs = slice(jc * COPY_N + jj * PSUM_N,
                                   jc * COPY_N + (jj + 1) * PSUM_N)
                        ps = slice(jj * PSUM_N, (jj + 1) * PSUM_N)
                        nc.tensor.matmul(
                            acc[:, ps], w_t[:, 0, :], x_t[:, 0, fs],
                            start=True, stop=False,
                        )
                        nc.tensor.matmul(
                            acc[:, ps], w_t[:, 1, :], x_t[:, 1, fs],
                            start=False, stop=True,
                        )
                    js = slice(jc * COPY_N, (jc + 1) * COPY_N)
                    if nco % 2 == 0:
                        nc.vector.tensor_copy(o_t[:, js], acc[:])
                    else:
                        nc.scalar.copy(o_t[:, js], acc[:])
                    nco += 1
                    if last_b:
                        # fine-grained stores on the (by now idle) SP ring
                        nc.sync.dma_start(
                            out_d[b, :, pos + jc * COPY_N:
                                  pos + (jc + 1) * COPY_N],
                            o_t[:, js],
                        )
                if not last_b:
                    # early stores ride the ACT ring while the SP ring is
                    # busy with loads; split big chunks in two so the
                    # store stream tracks compute
                    if cp > 2048:
                        nc.scalar.dma_start(
                            out_d[b, :, pos:pos + 2048], o_t[:, :2048]
                        )
                        nc.scalar.dma_start(
                            out_d[b, :, pos + 2048:pos + cp],
                            o_t[:, 2048:cp],
                        )
                    else:
                        nc.scalar.dma_start(
                            out_d[b, :, pos:pos + cp], o_t[:, :cp]
                        )
                pos += cp

    nc.compile()
    return nc


def _make_in_maps(x: np.ndarray, weight: np.ndarray) -> list[dict]:
    xf = np.ascontiguousarray(x, dtype=np.float32)
    # deinterleave even/odd phases: [bs, cin, 2, d/2], then fp8e3 encode
    xq = np.ascontiguousarray(
        xf.reshape(BS, CIN, P_OUT, 2).transpose(0, 1, 3, 2)
    ).astype(ml_dtypes.float8_e3m4)

    # wT[c, k, o] = weight[o, c, 0, k] / sqrt(cin)  (contiguous per-c line)
    wT = np.ascontiguousarray(
        np.transpose(weight[:, :, 0, :], (1, 2, 0)) / math.sqrt(CIN),
        dtype=np.float32,
    ).astype(ml_dtypes.bfloat16)

    return [
        {
            "xq": xq[i * B_PER_CORE:(i + 1) * B_PER_CORE],
            "wT": wT,
        }
        for i in range(N_CORES)
    ]


def kernel(x: np.ndarray, weight: np.ndarray) -> np.ndarray:
    if "nc" not in _cache:
        _cache["nc"] = _build()
    nc = _cache["nc"]
    in_maps = _make_in_maps(x, weight)
    res = run_bass_kernel_spmd(nc, in_maps, core_ids=list(range(N_CORES)))
    return np.concatenate(
        [r["out"].astype(np.float32) for r in res.results], axis=0
    )


# revision 12
# speedup vs baseline: 1.0029x; 1.0029x over previous
"""Non-overlapping Conv1d (kernel=2, stride=2) on 8 TRN2 NeuronCores.

out[b, o, p] = sum_{c,k} x[b, c, 2p+k] * w[o, c, k] / sqrt(cin)

Strategy: data-parallel over batch (4 batches per core), weight replicated.
Per batch: out[b] = W0 @ xe + W1 @ xo with the contraction over cin=128 on
the partition dim; xe/xo are the even/odd phases of x, deinterleaved on
the host so every on-chip access is contiguous.

Precision/traffic: the kernel is HBM-bound, so x is sent as fp8e3 (e3m4,
4 mantissa bits, 1 byte) and fed STRAIGHT into the PE as the moving
operand against bf16 stationary weights (mixed-dtype matmul measured
exact on HW, 1 col/cycle).  No on-chip dequant pass is needed.  Output is
stored bf16 and upconverted on the host.  End-to-end L2 error ~1.3e-2,
inside the 2e-2 gate.

Per-core HBM traffic: 4.19 MB x (fp8) + 4.19 MB out (bf16) = 8.39 MB.

Engines: SP ring (nc.sync) carries x loads + last-batch stores; ACT ring
carries early stores; weights ride the SWDGE (gpsimd) ring so they land
before the first matmul without delaying the x stream.  PSUM->SBUF
copies (paired banks, FD=1024) alternate DVE/ACT.
"""

import math
from contextlib import ExitStack

import numpy as np
import ml_dtypes

import concourse.bass as bass
import concourse.mybir as mybir
import concourse.tile as tile
from concourse import bacc
from concourse.bass_utils import run_bass_kernel_spmd

# Problem shape (hardcoded per contract)
BS, CIN, D = 32, 128, 8192
COUT = 128
N_CORES = 8
B_PER_CORE = BS // N_CORES          # 4
P_OUT = D // 2                      # 4096 output positions per (b, o)
PSUM_N = 512                        # fp32 PSUM bank limit = matmul free dim
COPY_N = 1024                       # positions per PSUM->SBUF copy (2 banks)

# per-batch chunk plans (positions): batch 0 starts fine-grained so the
# pipeline primes fast; the last batch's stores go per-COPY_N for a short
# tail
CHUNK_PLAN = [
    [1024, 1024, 2048],
    [4096],
    [4096],
    [4096],
]

_cache = {}


def _build():
    nc = bacc.Bacc("TRN2", target_bir_lowering=False, debug=False, num_devices=N_CORES)
    f32 = mybir.dt.float32
    bf16 = mybir.dt.bfloat16
    e3 = mybir.dt.float8e3

    x_d = nc.dram_tensor(
        "xq", [B_PER_CORE, CIN, 2, P_OUT], e3, kind="ExternalInput"
    ).ap()
    w_d = nc.dram_tensor("wT", [CIN, 2, COUT], bf16, kind="ExternalInput").ap()
    out_d = nc.dram_tensor(
        "out", [B_PER_CORE, COUT, P_OUT], bf16, kind="ExternalOutput"
    ).ap()

    with tile.TileContext(nc) as tc, ExitStack() as ctx:
        wpool = ctx.enter_context(tc.tile_pool(name="w", bufs=1))
        xpool = ctx.enter_context(tc.tile_pool(name="x", bufs=4))
        opool = ctx.enter_context(tc.tile_pool(name="o", bufs=4))
        ppool = ctx.enter_context(tc.tile_pool(name="p", bufs=4, space="PSUM"))

        # Weights first on the SP ring (host already stores them [c, k, o]
        # so the transfer is one contiguous 512B line per partition); they
        # land ~1 us after the ring warms, before the first matmul.
        w_t = wpool.tile([CIN, 2, COUT], bf16)
        nc.sync.dma_start(w_t[:], w_d)

        # flat chunk list: (batch, pos, cp, load_engine).  Batch 1 loads
        # ride the ACT ring (its trigger is issued before any copies queue
        # on the scalar engine), doubling load bandwidth early on.
        chunks = []
        for b in range(B_PER_CORE):
            pos = 0
            for cp in CHUNK_PLAN[b]:
                eng = nc.scalar if b == 1 else nc.sync
                chunks.append((b, pos, cp, eng))
                pos += cp

        PREFETCH = 4
        tiles = {}

        def issue_load(i):
            b, pos, cp, eng = chunks[i]
            x_t = xpool.tile([CIN, 2, 4096], e3, tag="x")
            eng.dma_start(x_t[:, :, :cp], x_d[b, :, :, pos:pos + cp])
            tiles[i] = x_t

        for i in range(min(PREFETCH, len(chunks))):
            issue_load(i)

        nco = 0  # running copy counter for DVE/ACT alternation
        for ci, (b, pos, cp, _eng) in enumerate(chunks):
            if ci + PREFETCH < len(chunks):
                issue_load(ci + PREFETCH)
            last_b = b == B_PER_CORE - 1
            x_t = tiles.pop(ci)
            if True:  # keep body indentation stable
                o_t = opool.tile([COUT, 4096], bf16, tag="o")
                for jc in range(cp // COPY_N):
                    acc = ppool.tile([COUT, COPY_N], f32, name="acc")
                    for jj in range(COPY_N // PSUM_N):
                        fs = slice(jc * COPY_N + jj * PSUM_N,
                                   jc * COPY_N + (jj + 1) * PSUM_N)
                        ps = slice(jj * PSUM_N, (jj + 1) * PSUM_N)
                        nc.tensor.matmul(
                            acc[:, ps], w_t[:, 0, :], x_t[:, 0, fs],
                            start=True, stop=False,
                        )
                        nc.tensor.matmul(
                            acc[:, ps], w_t[:, 1, :], x_t[:, 1, fs],
                            start=False, stop=True,
                        )
                    js = slice(jc * COPY_N, (jc + 1) * COPY_N)
                    if nco % 2 == 0:
                        nc.vector.tensor_copy(o_t[:, js], acc[:])
                    else:
                        nc.scalar.copy(o_t[:, js], acc[:])
                    nco += 1
                    if last_b:
                        # fine-grained stores on the (by now idle) SP ring
                        nc.sync.dma_start(
                            out_d[b, :, pos + jc * COPY_N:
                                  pos + (jc + 1) * COPY_N],
                            o_t[:, js],
                        )
                if not last_b:
                    # early stores ride the ACT ring while the SP ring is
                    # busy with loads; split big chunks in two so the
                    # store stream tracks compute
                    if cp > 2048:
                        nc.scalar.dma_start(
                            out_d[b, :, pos:pos + 2048], o_t[:, :2048]
                        )
                        nc.scalar.dma_start(
                            out_d[b, :, pos + 2048:pos + cp],
                            o_t[:, 2048:cp],
                        )
                    else:
                        nc.scalar.dma_start(
                            out_d[b, :, pos:pos + cp], o_t[:, :cp]
                        )

    nc.compile()
    return nc


def _make_in_maps(x: np.ndarray, weight: np.ndarray) -> list[dict]:
    xf = np.ascontiguousarray(x, dtype=np.float32)
    # deinterleave even/odd phases: [bs, cin, 2, d/2], then fp8e3 encode
    xq = np.ascontiguousarray(
        xf.reshape(BS, CIN, P_OUT, 2).transpose(0, 1, 3, 2)
    ).astype(ml_dtypes.float8_e3m4)

    # wT[c, k, o] = weight[o, c, 0, k] / sqrt(cin)  (contiguous per-c line)
    wT = np.ascontiguousarray(
        np.transpose(weight[:, :, 0, :], (1, 2, 0)) / math.sqrt(CIN),
        dtype=np.float32,
    ).astype(ml_dtypes.bfloat16)

    return [
        {
            "xq": xq[i * B_PER_CORE:(i + 1) * B_PER_CORE],
            "wT": wT,
        }
        for i in range(N_CORES)
    ]


def kernel(x: np.ndarray, weight: np.ndarray) -> np.ndarray:
    if "nc" not in _cache:
        _cache["nc"] = _build()
    nc = _cache["nc"]
    in_maps = _make_in_maps(x, weight)
    res = run_bass_kernel_spmd(nc, in_maps, core_ids=list(range(N_CORES)))
    return np.concatenate(
        [r["out"].astype(np.float32) for r in res.results], axis=0
    )


# revision 14
# speedup vs baseline: 1.0317x; 1.0286x over previous
"""Non-overlapping Conv1d (kernel=2, stride=2) on 8 TRN2 NeuronCores.

out[b, o, p] = sum_{c,k} x[b, c, 2p+k] * w[o, c, k] / sqrt(cin)

Strategy: data-parallel over batch (4 batches per core), weight replicated.
Per batch: out[b] = W0 @ xe + W1 @ xo with the contraction over cin=128 on
the partition dim; xe/xo are the even/odd phases of x, deinterleaved and
packed per-chunk on the host so every on-chip access is contiguous.

Precision/traffic: the kernel is HBM-bound, so x is sent as fp8e3 (e3m4,
4 mantissa bits, 1 byte) and fed STRAIGHT into the PE as the moving
operand against bf16 stationary weights (mixed-dtype matmul measured
exact on HW, 1 col/cycle).  No on-chip dequant pass.  Output is stored
bf16 and upconverted on the host.  End-to-end L2 error ~1.3e-2, inside
the 2e-2 gate.

Per-core HBM traffic: 4.19 MB x (fp8) + 4.19 MB out (bf16) = 8.39 MB.

DMA: x chunks ALTERNATE between the SP ring (nc.sync) and the ACT ring
(nc.scalar) in consumption order — the 16 SDMA engines round-robin
between queues per packet, so two rings deliver ~2x the load bandwidth.
Early stores ride the ACT ring behind its loads; the last batch stores
per-1024 on the SP ring (idle by then).  PSUM->SBUF copies (paired
banks, FD=1024) alternate DVE/ACT.
"""

import math
from contextlib import ExitStack

import numpy as np
import ml_dtypes

import concourse.bass as bass
import concourse.mybir as mybir
import concourse.tile as tile
from concourse import bacc
from concourse.bass_utils import run_bass_kernel_spmd

# Problem shape (hardcoded per contract)
BS, CIN, D = 32, 128, 8192
COUT = 128
N_CORES = 8
B_PER_CORE = BS // N_CORES          # 4
P_OUT = D // 2                      # 4096 output positions per (b, o)
PSUM_N = 512                        # fp32 PSUM bank limit = matmul free dim
COPY_N = 1024                       # positions per PSUM->SBUF copy (2 banks)

# global chunk plan (positions); batches are 4096 each:
# b0 = [1024, 1024, 2048], b1 = [2048, 2048], b2 = [2048, 2048],
# b3 = [2048, 2048]
CHUNK_PLAN = [
    [1024, 1024, 2048],
    [2048, 2048],
    [2048, 2048],
    [2048, 2048],
]

_cache = {}


def _chunk_list():
    out = []
    for b in range(B_PER_CORE):
        pos = 0
        for cp in CHUNK_PLAN[b]:
            out.append((b, pos, cp))
            pos += cp
    return out


def _build():
    nc = bacc.Bacc("TRN2", target_bir_lowering=False, debug=False, num_devices=N_CORES)
    f32 = mybir.dt.float32
    bf16 = mybir.dt.bfloat16
    e3 = mybir.dt.float8e3

    # x is packed on the host so each chunk is one contiguous [2*cp] line
    # per partition (uniform large DMA packets on both rings)
    x_d = nc.dram_tensor(
        "xq", [B_PER_CORE, CIN, D], e3, kind="ExternalInput"
    ).ap()
    w_d = nc.dram_tensor("wT", [CIN, 2, COUT], bf16, kind="ExternalInput").ap()
    out_d = nc.dram_tensor(
        "out", [B_PER_CORE, COUT, P_OUT], bf16, kind="ExternalOutput"
    ).ap()

    chunks = _chunk_list()

    with tile.TileContext(nc) as tc, ExitStack() as ctx:
        wpool = ctx.enter_context(tc.tile_pool(name="w", bufs=1))
        xpool = ctx.enter_context(tc.tile_pool(name="x", bufs=5))
        opool = ctx.enter_context(tc.tile_pool(name="o", bufs=4))
        ppool = ctx.enter_context(tc.tile_pool(name="p", bufs=4, space="PSUM"))

        # Weights first on the SP ring ([c, k, o] host layout: one
        # contiguous 512B line per partition)
        w_t = wpool.tile([CIN, 2, COUT], bf16)
        nc.sync.dma_start(w_t[:], w_d)

        PREFETCH = 5
        tiles = {}

        def issue_load(i):
            b, pos, cp, = chunks[i]
            eng = nc.sync if i % 2 == 0 else nc.scalar
            x_t = xpool.tile([CIN, 2, 2048], e3, tag="x")
            eng.dma_start(
                x_t[:, :, :cp],
                x_d[b, :, 2 * pos:2 * (pos + cp)].rearrange(
                    "c (k p) -> c k p", k=2
                ),
            )
            tiles[i] = x_t

        for i in range(min(PREFETCH, len(chunks))):
            issue_load(i)

        nco = 0  # running copy counter for DVE/ACT alternation
        for ci, (b, pos, cp) in enumerate(chunks):
            if ci + PREFETCH < len(chunks):
                issue_load(ci + PREFETCH)
            last_b = b == B_PER_CORE - 1
            x_t = tiles.pop(ci)
            o_t = opool.tile([COUT, 2048], bf16, tag="o")
            for jc in range(cp // COPY_N):
                acc = ppool.tile([COUT, COPY_N], f32, name="acc")
                for jj in range(COPY_N // PSUM_N):
                    fs = slice(jc * COPY_N + jj * PSUM_N,
                               jc * COPY_N + (jj + 1) * PSUM_N)
                    ps = slice(jj * PSUM_N, (jj + 1) * PSUM_N)
                    nc.tensor.matmul(
                        acc[:, ps], w_t[:, 0, :], x_t[:, 0, fs],
                        start=True, stop=False,
                    )
                    nc.tensor.matmul(
                        acc[:, ps], w_t[:, 1, :], x_t[:, 1, fs],
                        start=False, stop=True,
                    )
                js = slice(jc * COPY_N, (jc + 1) * COPY_N)
                if nco % 2 == 0:
                    nc.vector.tensor_copy(o_t[:, js], acc[:])
                else:
                    nc.scalar.copy(o_t[:, js], acc[:])
                nco += 1
                if last_b:
                    # fine-grained stores on the (by now idle) SP ring
                    nc.sync.dma_start(
                        out_d[b, :, pos + jc * COPY_N:
                              pos + (jc + 1) * COPY_N],
                        o_t[:, js],
                    )
            if not last_b:
                nc.scalar.dma_start(
                    out_d[b, :, pos:pos + cp], o_t[:, :cp]
                )

    nc.compile()
    return nc


def _make_in_maps(x: np.ndarray, weight: np.ndarray) -> list[dict]:
    xf = np.ascontiguousarray(x, dtype=np.float32)
    xq8 = xf.astype(ml_dtypes.float8_e3m4)
    # pack: per chunk, [xe(cp) ; xo(cp)] contiguous along d.  Chunk plans
    # differ by per-core batch slot (global batch g -> slot g % 4).
    packed = np.empty((BS, CIN, D), dtype=ml_dtypes.float8_e3m4)
    for slot, pos, cp in _chunk_list():
        src = xq8[slot::B_PER_CORE, :, 2 * pos:2 * (pos + cp)]
        packed[slot::B_PER_CORE, :, 2 * pos:2 * pos + cp] = src[:, :, 0::2]
        packed[slot::B_PER_CORE, :, 2 * pos + cp:2 * (pos + cp)] = src[:, :, 1::2]

    # wT[c, k, o] = weight[o, c, 0, k] / sqrt(cin)  (contiguous per-c line)
    wT = np.ascontiguousarray(
        np.transpose(weight[:, :, 0, :], (1, 2, 0)) / math.sqrt(CIN),
        dtype=np.float32,
    ).astype(ml_dtypes.bfloat16)

    return [
        {
            "xq": packed[i * B_PER_CORE:(i + 1) * B_PER_CORE],
            "wT": wT,
        }
        for i in range(N_CORES)
    ]


def kernel(x: np.ndarray, weight: np.ndarray) -> np.ndarray:
    if "nc" not in _cache:
        _cache["nc"] = _build()
    nc = _cache["nc"]
    in_maps = _make_in_maps(x, weight)
    res = run_bass_kernel_spmd(nc, in_maps, core_ids=list(range(N_CORES)))
    return np.concatenate(
        [r["out"].astype(np.float32) for r in res.results], axis=0
    )


# revision 17
# speedup vs baseline: 1.0435x; 1.0115x over previous
"""Non-overlapping Conv1d (kernel=2, stride=2) on 8 TRN2 NeuronCores.

out[b, o, p] = sum_{c,k} x[b, c, 2p+k] * w[o, c, k] / sqrt(cin)

Strategy: data-parallel over batch (4 batches per core), weight replicated.
Per batch: out[b] = W0 @ xe + W1 @ xo with the contraction over cin=128 on
the partition dim; xe/xo are the even/odd phases of x, deinterleaved and
packed per-chunk on the host so every on-chip access is contiguous.

Precision/traffic: the kernel is HBM-bound, so x is sent as fp8e3 (e3m4,
4 mantissa bits, 1 byte) and fed STRAIGHT into the PE as the moving
operand against bf16 stationary weights (mixed-dtype matmul measured
exact on HW, 1 col/cycle).  No on-chip dequant pass.  Output is stored
bf16 and upconverted on the host.  End-to-end L2 error ~1.3e-2, inside
the 2e-2 gate.

Per-core HBM traffic: 4.19 MB x (fp8) + 4.19 MB out (bf16) = 8.39 MB.

DMA: x chunks ALTERNATE between the SP ring (nc.sync) and the ACT ring
(nc.scalar) in consumption order — the 16 SDMA engines round-robin
between queues per packet, so two rings deliver ~2x the load bandwidth.
Early stores ride the ACT ring behind its loads; the last batch stores
per-1024 on the SP ring (idle by then).  PSUM->SBUF copies (paired
banks, FD=1024) alternate DVE/ACT.
"""

import math
from contextlib import ExitStack

import numpy as np
import ml_dtypes

import concourse.bass as bass
import concourse.mybir as mybir
import concourse.tile as tile
from concourse import bacc
from concourse.bass_utils import run_bass_kernel_spmd

# Problem shape (hardcoded per contract)
BS, CIN, D = 32, 128, 8192
COUT = 128
N_CORES = 8
B_PER_CORE = BS // N_CORES          # 4
P_OUT = D // 2                      # 4096 output positions per (b, o)
PSUM_N = 512                        # fp32 PSUM bank limit = matmul free dim
COPY_N = 1024                       # positions per PSUM->SBUF copy (2 banks)

# global chunk plan (positions); batches are 4096 each.  Small leading
# chunks prime the PE while the DMA rings are still ramping; small
# trailing chunks shorten the copy/store tail after the last matmul.
CHUNK_PLAN = [
    [512, 512, 1024, 2048],
    [2048, 2048],
    [2048, 2048],
    [2048, 1024, 512, 512],
]

_cache = {}


def _chunk_list():
    out = []
    for b in range(B_PER_CORE):
        pos = 0
        for cp in CHUNK_PLAN[b]:
            out.append((b, pos, cp))
            pos += cp
    return out


def _build():
    nc = bacc.Bacc("TRN2", target_bir_lowering=False, debug=False, num_devices=N_CORES)
    f32 = mybir.dt.float32
    bf16 = mybir.dt.bfloat16
    e3 = mybir.dt.float8e3

    # x is packed on the host so each chunk is one contiguous [2*cp] line
    # per partition (uniform large DMA packets on both rings)
    x_d = nc.dram_tensor(
        "xq", [B_PER_CORE, CIN, D], e3, kind="ExternalInput"
    ).ap()
    w_d = nc.dram_tensor("wT", [CIN, 2, COUT], bf16, kind="ExternalInput").ap()
    out_d = nc.dram_tensor(
        "out", [B_PER_CORE, COUT, P_OUT], bf16, kind="ExternalOutput"
    ).ap()

    chunks = _chunk_list()

    with tile.TileContext(nc) as tc, ExitStack() as ctx:
        wpool = ctx.enter_context(tc.tile_pool(name="w", bufs=1))
        xpool = ctx.enter_context(tc.tile_pool(name="x", bufs=5))
        opool = ctx.enter_context(tc.tile_pool(name="o", bufs=4))
        ppool = ctx.enter_context(tc.tile_pool(name="p", bufs=4, space="PSUM"))

        # Weights first on the SP ring ([c, k, o] host layout: one
        # contiguous 512B line per partition)
        w_t = wpool.tile([CIN, 2, COUT], bf16)
        nc.sync.dma_start(w_t[:], w_d)

        PREFETCH = 5
        tiles = {}

        def issue_load(i):
            b, pos, cp, = chunks[i]
            # alternate rings in consumption order; the last batch loads
            # stay on the SP ring (the ACT ring is store-heavy by then)
            eng = nc.sync if (i % 2 == 0 or i >= 8) else nc.scalar
            x_t = xpool.tile([CIN, 2, 2048], e3, tag="x")
            eng.dma_start(
                x_t[:, :, :cp],
                x_d[b, :, 2 * pos:2 * (pos + cp)].rearrange(
                    "c (k p) -> c k p", k=2
                ),
            )
            tiles[i] = x_t

        for i in range(min(PREFETCH, len(chunks))):
            issue_load(i)

        nco = 0  # running copy counter for DVE/ACT alternation
        for ci, (b, pos, cp) in enumerate(chunks):
            if ci + PREFETCH < len(chunks):
                issue_load(ci + PREFETCH)
            last_b = b == B_PER_CORE - 1
            x_t = tiles.pop(ci)
            o_t = opool.tile([COUT, 2048], bf16, tag="o")
            copy_n = min(COPY_N, cp)
            for jc in range(cp // copy_n):
                acc = ppool.tile([COUT, COPY_N], f32, name="acc")
                for jj in range(copy_n // PSUM_N):
                    fs = slice(jc * copy_n + jj * PSUM_N,
                               jc * copy_n + (jj + 1) * PSUM_N)
                    ps = slice(jj * PSUM_N, (jj + 1) * PSUM_N)
                    nc.tensor.matmul(
                        acc[:, ps], w_t[:, 0, :], x_t[:, 0, fs],
                        start=True, stop=False,
                    )
                    nc.tensor.matmul(
                        acc[:, ps], w_t[:, 1, :], x_t[:, 1, fs],
                        start=False, stop=True,
                    )
                js = slice(jc * copy_n, (jc + 1) * copy_n)
                if nco % 2 == 0:
                    nc.vector.tensor_copy(o_t[:, js], acc[:, :copy_n])
                else:
                    nc.scalar.copy(o_t[:, js], acc[:, :copy_n])
                nco += 1
                if last_b:
                    # fine-grained stores on the (by now idle) SP ring
                    nc.sync.dma_start(
                        out_d[b, :, pos + jc * copy_n:
                              pos + (jc + 1) * copy_n],
                        o_t[:, js],
                    )
            if not last_b:
                nc.scalar.dma_start(
                    out_d[b, :, pos:pos + cp], o_t[:, :cp]
                )

    nc.compile()
    return nc


def _make_in_maps(x: np.ndarray, weight: np.ndarray) -> list[dict]:
    xf = np.ascontiguousarray(x, dtype=np.float32)
    xq8 = xf.astype(ml_dtypes.float8_e3m4)
    # pack: per chunk, [xe(cp) ; xo(cp)] contiguous along d.  Chunk plans
    # differ by per-core batch slot (global batch g -> slot g % 4).
    packed = np.empty((BS, CIN, D), dtype=ml_dtypes.float8_e3m4)
    for slot, pos, cp in _chunk_list():
        src = xq8[slot::B_PER_CORE, :, 2 * pos:2 * (pos + cp)]
        packed[slot::B_PER_CORE, :, 2 * pos:2 * pos + cp] = src[:, :, 0::2]
        packed[slot::B_PER_CORE, :, 2 * pos + cp:2 * (pos + cp)] = src[:, :, 1::2]

    # wT[c, k, o] = weight[o, c, 0, k] / sqrt(cin)  (contiguous per-c line)
    wT = np.ascontiguousarray(
        np.transpose(weight[:, :, 0, :], (1, 2, 0)) / math.sqrt(CIN),
        dtype=np.float32,
    ).astype(ml_dtypes.bfloat16)

    return [
        {
            "xq": packed[i * B_PER_CORE:(i + 1) * B_PER_CORE],
            "wT": wT,
        }
        for i in range(N_CORES)
    ]


def kernel(x: np.ndarray, weight: np.ndarray) -> np.ndarray:
    if "nc" not in _cache:
        _cache["nc"] = _build()
    nc = _cache["nc"]
    in_maps = _make_in_maps(x, weight)
    res = run_bass_kernel_spmd(nc, in_maps, core_ids=list(range(N_CORES)))
    return np.concatenate(
        [r["out"].astype(np.float32) for r in res.results], axis=0
    )


# revision 22
# speedup vs baseline: 1.1158x; 1.0692x over previous
"""Non-overlapping Conv1d (kernel=2, stride=2) on 8 TRN2 NeuronCores.

out[b, o, p] = sum_{c,k} x[b, c, 2p+k] * w[o, c, k] / sqrt(cin)

Strategy: data-parallel over batch (4 batches per core), weight replicated.
Per batch: out[b] = W0 @ xe + W1 @ xo with the contraction over cin=128 on
the partition dim; xe/xo are the even/odd phases of x, deinterleaved and
packed per-chunk on the host so every on-chip access is contiguous.

Precision/traffic: the kernel is HBM-bound, so x is sent as fp8e3 (e3m4,
4 mantissa bits, 1 byte) and fed STRAIGHT into the PE as the moving
operand against bf16 stationary weights (mixed-dtype matmul measured
exact on HW, 1 col/cycle).  No on-chip dequant pass.  Output is stored
bf16 and upconverted on the host.  End-to-end L2 error ~1.3e-2, inside
the 2e-2 gate.

Per-core HBM traffic: 4.19 MB x (fp8) + 4.19 MB out (bf16) = 8.39 MB.

DMA: x chunks ALTERNATE between the SP ring (nc.sync) and the ACT ring
(nc.scalar) in consumption order — the 16 SDMA engines round-robin
between queues per packet, so two rings deliver ~2x the load bandwidth.
Early stores ride the ACT ring behind its loads; the last batch stores
per-1024 on the SP ring (idle by then).  PSUM->SBUF copies (paired
banks, FD=1024) alternate DVE/ACT.
"""

import math
from contextlib import ExitStack

import numpy as np
import ml_dtypes

import concourse.bass as bass
import concourse.mybir as mybir
import concourse.tile as tile
from concourse import bacc
from concourse.bass_utils import run_bass_kernel_spmd

# Problem shape (hardcoded per contract)
BS, CIN, D = 32, 128, 8192
COUT = 128
N_CORES = 8
B_PER_CORE = BS // N_CORES          # 4
P_OUT = D // 2                      # 4096 output positions per (b, o)
PSUM_N = 512                        # fp32 PSUM bank limit = matmul free dim
COPY_N = 1024                       # positions per PSUM->SBUF copy (2 banks)

# global chunk plan (positions); batches are 4096 each.  Small leading
# chunks prime the PE while the DMA rings are still ramping; small
# trailing chunks shorten the copy/store tail after the last matmul.
CHUNK_PLAN = [
    [256, 256, 512, 1024, 2048],
    [2048, 2048],
    [2048, 2048],
    [2048, 1024, 512, 512],
]

_cache = {}


def _chunk_list():
    out = []
    for b in range(B_PER_CORE):
        pos = 0
        for cp in CHUNK_PLAN[b]:
            out.append((b, pos, cp))
            pos += cp
    return out


def _build():
    nc = bacc.Bacc("TRN2", target_bir_lowering=False, debug=False, num_devices=N_CORES)
    f32 = mybir.dt.float32
    bf16 = mybir.dt.bfloat16
    e3 = mybir.dt.float8e3

    # x is packed on the host so each chunk is one contiguous [2*cp] line
    # per partition (uniform large DMA packets on both rings)
    x_d = nc.dram_tensor(
        "xq", [B_PER_CORE, CIN, D], e3, kind="ExternalInput"
    ).ap()
    w_d = nc.dram_tensor("wT", [CIN, 2, COUT], bf16, kind="ExternalInput").ap()
    out_d = nc.dram_tensor(
        "out", [B_PER_CORE, COUT, P_OUT], bf16, kind="ExternalOutput"
    ).ap()

    chunks = _chunk_list()

    with tile.TileContext(nc) as tc, ExitStack() as ctx:
        wpool = ctx.enter_context(tc.tile_pool(name="w", bufs=1))
        xpool = ctx.enter_context(tc.tile_pool(name="x", bufs=6))
        opool = ctx.enter_context(tc.tile_pool(name="o", bufs=6))
        ppool = ctx.enter_context(tc.tile_pool(name="p", bufs=4, space="PSUM"))

        PREFETCH = 6
        tiles = {}
        w_t = wpool.tile([CIN, 2, COUT], bf16)

        def issue_load(i):
            b, pos, cp, = chunks[i]
            # alternate rings in consumption order; the last batch loads
            # stay on the SP ring (the ACT ring is store-heavy by then)
            eng = nc.sync if (i % 2 == 0 or i >= 9) else nc.scalar
            x_t = xpool.tile([CIN, 2, 2048], e3, tag="x")
            eng.dma_start(
                x_t[:, :, :cp],
                x_d[b, :, 2 * pos:2 * (pos + cp)].rearrange(
                    "c (k p) -> c k p", k=2
                ),
            )
            tiles[i] = x_t

        # first chunk's load goes out first (it gates the first matmul);
        # the tiny weight load follows on the same ring
        issue_load(0)
        nc.sync.dma_start(w_t[:], w_d)
        for i in range(1, min(PREFETCH, len(chunks))):
            issue_load(i)

        nco = 0  # running copy counter for DVE/ACT alternation
        for ci, (b, pos, cp) in enumerate(chunks):
            if ci + PREFETCH < len(chunks):
                issue_load(ci + PREFETCH)
            last_b = b == B_PER_CORE - 1
            x_t = tiles.pop(ci)
            o_t = opool.tile([COUT, 2048], bf16, tag="o")
            copy_n = min(COPY_N, cp)
            mm_n = min(PSUM_N, copy_n)
            for jc in range(cp // copy_n):
                acc = ppool.tile([COUT, COPY_N], f32, name="acc")
                for jj in range(copy_n // mm_n):
                    fs = slice(jc * copy_n + jj * mm_n,
                               jc * copy_n + (jj + 1) * mm_n)
                    ps = slice(jj * mm_n, (jj + 1) * mm_n)
                    nc.tensor.matmul(
                        acc[:, ps], w_t[:, 0, :], x_t[:, 0, fs],
                        start=True, stop=False,
                    )
                    nc.tensor.matmul(
                        acc[:, ps], w_t[:, 1, :], x_t[:, 1, fs],
                        start=False, stop=True,
                    )
                js = slice(jc * copy_n, (jc + 1) * copy_n)
                if nco % 2 == 0:
                    nc.vector.tensor_copy(o_t[:, js], acc[:, :copy_n])
                else:
                    nc.scalar.copy(o_t[:, js], acc[:, :copy_n])
                nco += 1
                if last_b:
                    # fine-grained stores on the (by now idle) SP ring
                    nc.sync.dma_start(
                        out_d[b, :, pos + jc * copy_n:
                              pos + (jc + 1) * copy_n],
                        o_t[:, js],
                    )
            if not last_b:
                nc.scalar.dma_start(
                    out_d[b, :, pos:pos + cp], o_t[:, :cp]
                )

    nc.compile()
    return nc


def _make_in_maps(x: np.ndarray, weight: np.ndarray) -> list[dict]:
    xf = np.ascontiguousarray(x, dtype=np.float32)
    xq8 = xf.astype(ml_dtypes.float8_e3m4)
    # pack: per chunk, [xe(cp) ; xo(cp)] contiguous along d.  Chunk plans
    # differ by per-core batch slot (global batch g -> slot g % 4).
    packed = np.empty((BS, CIN, D), dtype=ml_dtypes.float8_e3m4)
    for slot, pos, cp in _chunk_list():
        src = xq8[slot::B_PER_CORE, :, 2 * pos:2 * (pos + cp)]
        packed[slot::B_PER_CORE, :, 2 * pos:2 * pos + cp] = src[:, :, 0::2]
        packed[slot::B_PER_CORE, :, 2 * pos + cp:2 * (pos + cp)] = src[:, :, 1::2]

    # wT[c, k, o] = weight[o, c, 0, k] / sqrt(cin)  (contiguous per-c line)
    wT = np.ascontiguousarray(
        np.transpose(weight[:, :, 0, :], (1, 2, 0)) / math.sqrt(CIN),
        dtype=np.float32,
    ).astype(ml_dtypes.bfloat16)

    return [
        {
            "xq": packed[i * B_PER_CORE:(i + 1) * B_PER_CORE],
            "wT": wT,
        }
        for i in range(N_CORES)
    ]


def kernel(x: np.ndarray, weight: np.ndarray) -> np.ndarray:
    if "nc" not in _cache:
        _cache["nc"] = _build()
    nc = _cache["nc"]
    in_maps = _make_in_maps(x, weight)
    res = run_bass_kernel_spmd(nc, in_maps, core_ids=list(range(N_CORES)))
    return np.concatenate(
        [r["out"].astype(np.float32) for r in res.results], axis=0
    )
